# revision 43
# baseline (speedup 1.0000x reference)
"""Fused pre-norm attention kernel for Trainium2, sharded over 8 NeuronCores.

Problem: out = (LayerNorm(x) @ w_qkv -> multi-head attention) @ w_out
  x [4, 2048, 512], 8 heads x 64 dim, fp32.

Sharding: core c computes batch b = c//2 with head group g = c%2 (4 heads).
Each core produces a partial output [2048, 512]; the host sums the two
partials per batch and adds the (exact) v-bias @ w_out correction row.

Per-core kernel (v2): all-bf16 PE paths, dual-engine softmax exp.
  1. LayerNorm token-major on DVE (bn_stats); normalized tile written bf16;
     4 PE transposes land in ONE bf16 PSUM bank, staged to xn^T by a single
     ACT copy per tile. v-projection per token tile on the PE (v-bias is
     folded into the host-side epilogue: P@(V+1 b^T) = P@V + den*b^T, so the
     normalized output only needs +b, i.e. +b@w_out after out-projection).
  2. q/k projections bf16: q(pair0,slice0) + all pair-0 k upfront; the
     remaining 11 groups stream into the attention loop at <=2 steps/kb.
  3. Attention in S^T layout, head-paired (rows 0-63 / 64-127 of the PE run
     the two heads' K=64 S^T matmuls concurrently). Softmax exp is split
     across TWO engines: kb%5==2 goes to the DVE as a one-pass Schraudolph
     bit-trick (tensor_scalar mult+add -> int16 = bf16 bits of
     2^((s*SCALE-c)*log2e)), the rest to ACT's exact exp. P@V consumes
     key-block kb-2 (depth-2 software pipeline -- keeps the S matmul of the
     next block from head-of-line blocking behind a P@V that still waits on
     its exp). A ones-column appended to v gives the softmax denominator.
  4. Normalization: oc staged bf16 (fast PSUM release); denominator row
     staged to partition 0 then reciprocal_approx_fast (the custom op
     mishandles base_partition>0); DRAM-bounce partition-broadcast; lazy
     multiply writes oT bf16.
  5. Out-projection bf16, pinned on a covering attention flush so it
     overlaps late attention without stalling the PE queue.
"""

import os
import sys
from contextlib import ExitStack

import numpy as np

for _p in ("/opt/trn_rl_repo",):
    if _p not in sys.path and os.path.isdir(_p):
        sys.path.insert(0, _p)

import concourse.bacc as bacc
import concourse.bass as bass
import concourse.mybir as mybir
import concourse.tile as tile
from concourse.bass_utils import run_bass_kernel_spmd
from concourse.masks import make_identity

F32 = mybir.dt.float32
BF16 = mybir.dt.bfloat16
I16 = mybir.dt.int16
I32 = mybir.dt.int32
AF = mybir.ActivationFunctionType

N_CORES = 8
B, N, D = 4, 2048, 512
H_PER_CORE = 4
DH = 64
GCOLS = H_PER_CORE * DH          # 256 columns per head-group
WCOLS = 3 * GCOLS                # 768 qkv columns per core
SCALE = DH ** -0.5
EPS = 1e-5
P = 128                          # SBUF partitions
NT = N // P                      # 16 token tiles
KT = D // P                      # 4 feature (contraction) tiles
QTW = 512                        # query-slice width for attention
NQT = N // QTW                   # 4 query slices

# Schraudolph bf16-space exp: i16 = round(s*SCALE*log2e*128 + 127*128 - C)
# C centers the one-sided (1+f)/2^f error so DVE-exp'd key blocks are not
# systematically overweighted vs ACT-exp'd ones in the softmax.
EXP_A = float(SCALE * 128.0 / np.log(2.0))
EXP_C = float(os.environ.get("BASS_EXP_C", "7.33"))
EXP_B = float(127 * 128) - EXP_C
# kb % DVE_DIV == DVE_MOD routes that key block's exp to the DVE
DVE_DIV = int(os.environ.get("BASS_DVE_DIV", "5"))
DVE_MOD = int(os.environ.get("BASS_DVE_MOD", "2"))


def _build_nc():
    nc = bacc.Bacc(None)
    x_d = nc.declare_dram_parameter("x", [N, D], F32, isOutput=False)
    wqkv_d = nc.declare_dram_parameter("wqkv", [D, WCOLS], BF16, isOutput=False)
    bqkv_d = nc.declare_dram_parameter("bqkv", [WCOLS, 1], F32, isOutput=False)
    wout_d = nc.declare_dram_parameter("wout", [GCOLS, D], BF16, isOutput=False)
    out_d = nc.declare_dram_parameter("out", [N, D], F32, isOutput=True)

    with tile.TileContext(nc, pool_alloc_mode="queue") as tc, ExitStack() as ctx:
        singles = ctx.enter_context(tc.tile_pool(name="singles", bufs=1))
        xin = ctx.enter_context(tc.tile_pool(name="xin", bufs=6))
        xnp = ctx.enter_context(tc.tile_pool(name="xnp", bufs=4))
        stats = ctx.enter_context(tc.tile_pool(name="stats", bufs=4))
        pP = ctx.enter_context(tc.tile_pool(name="pP", bufs=4))
        pPi = ctx.enter_context(tc.tile_pool(name="pPi", bufs=3))
        smalls = ctx.enter_context(tc.tile_pool(name="smalls", bufs=6))
        rbp = ctx.enter_context(tc.tile_pool(name="rbp", bufs=3))
        outp = ctx.enter_context(tc.tile_pool(name="outp", bufs=3))
        psV = ctx.enter_context(tc.tile_pool(name="psV", bufs=2, space="PSUM"))
        psS = ctx.enter_context(tc.tile_pool(name="psS", bufs=3, space="PSUM"))
        dscr = ctx.enter_context(tc.tile_pool(name="dscr", bufs=6, space="DRAM"))

        identf = singles.tile([P, P], F32)
        make_identity(nc, identf)
        ident = singles.tile([P, P], BF16)
        nc.vector.tensor_copy(out=ident, in_=identf)

        # persistent SBUF tensors
        xT = singles.tile([P, KT, N], BF16)             # xn^T  [feat, token]
        qkT = singles.tile([P, 4, N], BF16)             # [qT(2 tiles), kT(2 tiles)]
        v_aug = singles.tile([P, NT, H_PER_CORE, DH + 1], BF16)
        oT = singles.tile([P, 2, N], BF16)              # O^T rows (4 heads x 64)
        w_sb = singles.tile([P, KT, WCOLS], BF16)
        bias_sb = singles.tile([P, 6], F32)
        oc_all = singles.tile([DH, NQT * H_PER_CORE, QTW], BF16)
        wout_sb = singles.tile([P, 2, D], BF16)

        # ones columns of v_aug
        ones_sb = singles.tile([P, 1], F32)
        nc.vector.memset(ones_sb, 1.0)
        nc.vector.tensor_copy(
            out=v_aug[:, :, :, DH : DH + 1],
            in_=ones_sb.to_broadcast((P, NT, H_PER_CORE, 1)),
        )

        # PE matmuls accept only ONE sync wait command. Sacrificial ldweights
        # ops (no PSUM output, single dependency each) make the PE observe
        # fresh semaphore ticks so real matmuls keep to one wait.
        def pe_observe(ap):
            nc.tensor.ldweights(ap.bitcast(BF16))

        pe_observe(ident[:, 0:1])

        # ---- q/k projection group builder (used by phases A/B/C) -----------
        def qk_group_steps(mi, nt):
            ps = psS.tile([P, QTW], F32, tag="s")
            for kt in range(KT):
                yield lambda kt=kt, ps=ps: nc.tensor.matmul(
                    ps,
                    w_sb[:, kt, mi * P : (mi + 1) * P],
                    xT[:, kt, nt * QTW : (nt + 1) * QTW],
                    start=(kt == 0),
                    stop=(kt == KT - 1),
                )

            def bias_and_observe(ps=ps, mi=mi, nt=nt):
                nc.vector.tensor_scalar(
                    out=qkT[:, mi, nt * QTW : (nt + 1) * QTW],
                    in0=ps,
                    scalar1=bias_sb[:, mi : mi + 1],
                    scalar2=None,
                    op0=mybir.AluOpType.add,
                )
                pe_observe(qkT[:, mi, (nt + 1) * QTW - 1 : (nt + 1) * QTW])

            yield bias_and_observe

        # ---- Phase B/C machinery (defined early: phase A pumps attention) --
        def steps_of(groups):
            for mi, nt in groups:
                yield from qk_group_steps(mi, nt)

        _deferred = steps_of([])  # all groups run in phase A PE slack

        # group (mi, nt) only needs xT tokens [nt*512, (nt+1)*512), complete
        # after tile 4nt+3: spread all 16 groups across phase A
        QK_SCHED = {
            3: [(0, 0), (2, 0)], 4: [(3, 0)], 5: [(1, 0)],
            7: [(2, 1), (0, 1)], 8: [(3, 1)], 9: [(1, 1)],
            11: [(2, 2), (0, 2)], 12: [(3, 2)], 13: [(1, 2)],
            15: [(2, 3), (0, 3), (3, 3), (1, 3)],
        }

        last_flush = {}
        norm_pending = []
        phase_a_done = [False]

        def make_norm_chain(hp, qt, h0, h1, po0, po1, qs):
            def chain():
                # normalize both heads: bf16 staging copy + fast reciprocal
                # release the PSUM slots promptly; the DMA-broadcast/mul
                # chain lags by design and only gates phase D.
                for h, po in ((h0, po0), (h1, po1)):
                    u = h * NQT + qt
                    r0 = (h % 2) * DH
                    nc.vector.tensor_copy(out=oc_all[:, u, :], in_=po[0:DH, :])
                    den = smalls.tile([1, QTW], F32)
                    nc.vector.tensor_copy(out=den, in_=po[DH : DH + 1, :])
                    r = smalls.tile([1, QTW], F32)
                    nc.vector.reciprocal_approx_fast(out=r, in_=den)
                    rd = dscr.tile([1, QTW], F32)
                    nc.sync.dma_start(out=rd, in_=r)
                    rb = rbp.tile([DH, QTW], F32)
                    nc.sync.dma_start(out=rb, in_=rd.to_broadcast((DH, QTW)))
                    nc.vector.tensor_mul(
                        out=oT[r0 : r0 + DH, h // 2, qs],
                        in0=oc_all[:, u, :],
                        in1=rb,
                    )

            return chain

        # attention (S^T layout, head-paired), as a generator yielding after
        # each key-block iteration so phase A can pump unit (0,0) early
        def attention_gen():
            for hp in range(2):
                h0, h1 = 2 * hp, 2 * hp + 1
                mi_q, mi_k = hp, 2 + hp
                for qt in range(NQT):
                    qs = slice(qt * QTW, (qt + 1) * QTW)
                    po0 = psV.tile([DH + 1, QTW], F32, tag="po")
                    po1 = psV.tile([DH + 1, QTW], F32, tag="po")
                    pend = []

                    def pv_flush(pkb, ppT, stop, hp=hp, h0=h0, h1=h1,
                                 po0=po0, po1=po1):
                        if pkb == 0:
                            pe_observe(ppT[:, 0, 0:1])
                        nc.tensor.matmul(
                            po0, v_aug[:, pkb, h0, :], ppT[:, 0, :],
                            start=(pkb == 0), stop=stop,
                        )
                        mm = nc.tensor.matmul(
                            po1, v_aug[:, pkb, h1, :], ppT[:, 1, :],
                            start=(pkb == 0), stop=stop,
                        )
                        if hp == 0 and phase_a_done[0]:
                            for _ in range(2):
                                step = next(_deferred, None)
                                if step is not None:
                                    step()
                        return mm

                    for kb in range(NT):
                        if kb == 1 and norm_pending:
                            # previous unit's normalize chain, deferred past
                            # the unit boundary so its DVE ops don't delay
                            # this unit's pipeline ramp
                            norm_pending.pop(0)()
                        ks = slice(kb * P, (kb + 1) * P)
                        ps_s = psS.tile([P, 2, QTW], F32, tag="s")
                        nc.tensor.matmul(
                            ps_s[:, 0, :],
                            qkT[0:DH, mi_k, ks],
                            qkT[0:DH, mi_q, qs],
                            start=True,
                            stop=True,
                        )
                        nc.tensor.matmul(
                            ps_s[:, 1, :],
                            qkT[DH:P, mi_k, ks],
                            qkT[DH:P, mi_q, qs],
                            start=True,
                            stop=True,
                        )
                        if kb % DVE_DIV == DVE_MOD:
                            # DVE Schraudolph: i16 bits of 2^((s*SCALE-c)*log2e)
                            pTi = pPi.tile([P, 2, QTW], I16)
                            nc.vector.tensor_scalar(
                                out=pTi,
                                in0=ps_s,
                                scalar1=EXP_A,
                                scalar2=EXP_B,
                                op0=mybir.AluOpType.mult,
                                op1=mybir.AluOpType.add,
                            )
                            pT = pTi.bitcast(BF16)
                        else:
                            pT = pP.tile([P, 2, QTW], BF16)
                            nc.scalar.activation(
                                out=pT, in_=ps_s, func=AF.Exp, scale=SCALE
                            )
                        if len(pend) == 3:
                            pkb, ppT = pend.pop(0)
                            pv_flush(pkb, ppT, stop=False)
                        pend.append((kb, pT))
                        yield
                    pv_flush(*pend.pop(0), stop=False)
                    pv_flush(*pend.pop(0), stop=False)
                    last_att_mm = pv_flush(*pend.pop(0), stop=True)
                    if hp == 1:
                        last_flush[qt] = last_att_mm
                    norm_pending.append(
                        make_norm_chain(hp, qt, h0, h1, po0, po1, qs)
                    )

        _att = attention_gen()

        def pump(n):
            for _ in range(n):
                if next(_att, "END") == "END":
                    break

        # ---- Phase A: LayerNorm + transpose + v projection -----------------
        for tt in range(NT):
            x_tile = xin.tile([P, D], F32)
            nc.sync.dma_start(out=x_tile, in_=x_d[tt * P : (tt + 1) * P, :])
            if tt == 0:
                # weight DMAs issued after the first x tile so their
                # descriptor generation doesn't delay the LayerNorm start;
                # split per kt so no single queue carries the full 768KB
                for kt in range(KT):
                    nc.sync.dma_start(
                        out=w_sb[:, kt, :],
                        in_=wqkv_d[kt * P : (kt + 1) * P, :],
                    )
                nc.sync.dma_start(
                    out=bias_sb, in_=bqkv_d[:, :].rearrange("(t p) o -> p (t o)", p=P)
                )
                for ki in range(2):
                    nc.sync.dma_start(
                        out=wout_sb[:, ki, :],
                        in_=wout_d[ki * P : (ki + 1) * P, :],
                    )
                for kt in range(KT):
                    pe_observe(w_sb[:, kt, 2 * GCOLS : 2 * GCOLS + 1])
                pe_observe(wout_sb[:, 0, 0:1])
                pe_observe(wout_sb[:, 1, 0:1])
            st = stats.tile([P, nc.vector.BN_STATS_DIM], F32)
            nc.vector.bn_stats(out=st, in_=x_tile)
            mv = stats.tile([P, nc.vector.BN_AGGR_DIM], F32)
            nc.vector.bn_aggr(out=mv, in_=st)
            # rstd = rsqrt(var+eps) entirely on the DVE (quake seed + one
            # Newton step, ~0.2% -- under the bf16 rounding already applied
            # to xn). Keeping Sqrt off ACT means phase A only uses Copy/Exp,
            # all in ACT's exp_and_others table set: interleaving the pumped
            # attention exps causes no ~2.9us ACT_TABLE_LOAD ping-pong.
            ve = stats.tile([P, 1], F32)
            nc.vector.tensor_scalar(
                out=ve, in0=mv[:, 1:2], scalar1=EPS, scalar2=None,
                op0=mybir.AluOpType.add,
            )
            y0i = stats.tile([P, 1], I32)
            nc.vector.tensor_scalar(
                out=y0i, in0=ve.bitcast(I32), scalar1=1, scalar2=None,
                op0=mybir.AluOpType.logical_shift_right,
            )
            nc.vector.tensor_scalar(
                out=y0i, in0=y0i, scalar1=-1, scalar2=0x5F3759DF,
                op0=mybir.AluOpType.mult, op1=mybir.AluOpType.add,
            )
            y0 = y0i.bitcast(F32)
            t1 = stats.tile([P, 1], F32)
            nc.vector.tensor_mul(out=t1, in0=y0, in1=y0)
            nc.vector.tensor_mul(out=t1, in0=t1, in1=ve)
            nc.vector.tensor_scalar(
                out=t1, in0=t1, scalar1=-0.5, scalar2=1.5,
                op0=mybir.AluOpType.mult, op1=mybir.AluOpType.add,
            )
            y1 = stats.tile([P, 1], F32)
            nc.vector.tensor_mul(out=y1, in0=y0, in1=t1)
            t2 = stats.tile([P, 1], F32)
            nc.vector.tensor_mul(out=t2, in0=y1, in1=y1)
            nc.vector.tensor_mul(out=t2, in0=t2, in1=ve)
            nc.vector.tensor_scalar(
                out=t2, in0=t2, scalar1=-0.5, scalar2=1.5,
                op0=mybir.AluOpType.mult, op1=mybir.AluOpType.add,
            )
            rstd = stats.tile([P, 1], F32)
            nc.vector.tensor_mul(out=rstd, in0=y1, in1=t2)
            xn = xnp.tile([P, D], BF16)
            nc.vector.tensor_scalar(
                out=xn,
                in0=x_tile,
                scalar1=mv[:, 0:1],
                scalar2=rstd,
                op0=mybir.AluOpType.subtract,
                op1=mybir.AluOpType.mult,
            )
            # 4 bf16 transposes into ONE PSUM bank (borrowing the S pool,
            # idle in phase A), one ACT copy to xT
            ps4 = psS.tile([P, KT, P], BF16, tag="s")
            for ft in range(KT):
                nc.tensor.transpose(
                    ps4[:, ft, :], xn[:, ft * P : (ft + 1) * P], ident
                )
            nc.scalar.copy(
                out=xT[:, :, tt * P : (tt + 1) * P].rearrange("p k t -> p k t"),
                in_=ps4,
            )
            pe_observe(xT[:, KT - 1, tt * P + P - 1 : tt * P + P])
            # v projection (no bias: folded into the host epilogue)
            ps = psS.tile([P, GCOLS], F32, tag="s")
            for kt in range(KT):
                nc.tensor.matmul(
                    ps,
                    xT[:, kt, tt * P : (tt + 1) * P],
                    w_sb[:, kt, 2 * GCOLS : 3 * GCOLS],
                    start=(kt == 0),
                    stop=(kt == KT - 1),
                )
            nc.vector.tensor_copy(
                out=v_aug[:, tt, :, 0:DH],
                in_=ps.rearrange("p (h d) -> p h d", h=H_PER_CORE),
            )
            # q/k groups interleaved into phase A PE slack
            for mi, nt in QK_SCHED.get(tt, ()):
                for step in qk_group_steps(mi, nt):
                    step()
            # pump one attention iteration of unit (0,0) per tile: kb=tt-4
            # only needs k(nt=(tt-4)//4) (ready at tt>=4nt+3) and v_aug
            # tiles <= tt-4, both already emitted
            if tt >= 4:
                pump(1)

        # ---- Phase C: finish the attention stream --------------------------
        phase_a_done[0] = True
        for _ in _att:
            pass

        while norm_pending:
            norm_pending.pop(0)()

        # Keep the PE HAM-warm across the normalize-chain tail.
        for wk in range(8):
            ps = psS.tile([P, QTW], F32, tag="s")
            nc.tensor.matmul(
                ps,
                qkT[0:DH, 0, 0:P],
                qkT[0:DH, 0, 0:QTW],
                start=True,
                stop=True,
            )

        # PE observes the final oT write tick before the out-projection
        pe_observe(oT[0:DH, 1, N - 1 : N])

        # ---- Phase D: out projection -----------------------------------
        ob_hist = []
        for tt in range(NT):
            if len(ob_hist) >= 2:
                pe_observe(ob_hist[-2][:, 0:1])
            ps = psS.tile([P, D], F32, tag="s")
            for ki in range(2):
                mm = nc.tensor.matmul(
                    ps,
                    oT[:, ki, tt * P : (tt + 1) * P],
                    wout_sb[:, ki, :],
                    start=(ki == 0),
                    stop=(ki == 1),
                )
                pin = last_flush[min(tt // NQT + 1, NQT - 1)]
                tile.add_dep_helper(
                    mm.ins, pin.ins, sync=False,
                    reason="phase D after covering attention flush",
                )
            ob = outp.tile([P, D], F32)
            nc.vector.tensor_copy(out=ob, in_=ps)
            ob_hist.append(ob)
            nc.sync.dma_start(out=out_d[tt * P : (tt + 1) * P, :], in_=ob)

    nc.compile()
    return nc


_NC_CACHE = {}
last_results = None  # BassKernelResults of the most recent run (for test.py)


def _get_nc():
    key = (EXP_C, DVE_DIV, DVE_MOD)
    if key not in _NC_CACHE:
        _NC_CACHE[key] = _build_nc()
    return _NC_CACHE[key]


def _to_bf16(a):
    import ml_dtypes

    return np.ascontiguousarray(a.astype(ml_dtypes.bfloat16))


def kernel(x, gamma, beta, w_qkv, w_out):
    global last_results
    x = np.ascontiguousarray(np.asarray(x, dtype=np.float32))
    gamma = np.asarray(gamma, dtype=np.float32)
    beta = np.asarray(beta, dtype=np.float32)
    w_qkv = np.asarray(w_qkv, dtype=np.float32)
    w_out = np.asarray(w_out, dtype=np.float32)

    # fold gamma/beta into the projection (exact algebra)
    wp = gamma[:, None] * w_qkv                      # [512, 1536]
    bp = beta @ w_qkv                                # [1536]

    in_maps = []
    for c in range(N_CORES):
        b = c // 2
        g = c % 2
        sl = [slice(s * D + g * GCOLS, s * D + (g + 1) * GCOLS) for s in range(3)]
        wg = np.concatenate([wp[:, s] for s in sl], axis=1)          # [512, 768]
        bg = np.concatenate([bp[s] for s in sl])[:, None]            # [768, 1]
        wo = w_out[g * GCOLS : (g + 1) * GCOLS, :]                   # [256, 512]
        in_maps.append(
            {
                "x": np.ascontiguousarray(x[b]),
                "wqkv": _to_bf16(wg),
                "bqkv": np.ascontiguousarray(bg.astype(np.float32)),
                "wout": _to_bf16(wo),
            }
        )

    nc = _get_nc()
    last_results = run_bass_kernel_spmd(nc, in_maps, list(range(N_CORES)))
    outs = [m["out"] for m in last_results.results]
    # v-bias epilogue: P@(V + 1 b^T) = P@V + den*b^T, so after normalization
    # the correction is simply +b_v @ w_out on every token row (exact).
    vb_corr = (bp[2 * D :] @ w_out).astype(np.float32)   # [512]
    out = np.stack([outs[2 * b] + outs[2 * b + 1] + vb_corr for b in range(B)])
    return np.ascontiguousarray(out.astype(np.float32))


# revision 44
# speedup vs baseline: 1.0180x; 1.0180x over previous
"""Fused pre-norm attention kernel for Trainium2, sharded over 8 NeuronCores.

Problem: out = (LayerNorm(x) @ w_qkv -> multi-head attention) @ w_out
  x [4, 2048, 512], 8 heads x 64 dim, fp32.

Sharding: core c computes batch b = c//2 with head group g = c%2 (4 heads).
Each core produces a partial output [2048, 512]; the host sums the two
partials per batch and adds the (exact) v-bias @ w_out correction row.

Per-core kernel (v2): all-bf16 PE paths, dual-engine softmax exp.
  1. LayerNorm token-major on DVE (bn_stats); normalized tile written bf16;
     4 PE transposes land in ONE bf16 PSUM bank, staged to xn^T by a single
     ACT copy per tile. v-projection per token tile on the PE (v-bias is
     folded into the host-side epilogue: P@(V+1 b^T) = P@V + den*b^T, so the
     normalized output only needs +b, i.e. +b@w_out after out-projection).
  2. q/k projections bf16: q(pair0,slice0) + all pair-0 k upfront; the
     remaining 11 groups stream into the attention loop at <=2 steps/kb.
  3. Attention in S^T layout, head-paired (rows 0-63 / 64-127 of the PE run
     the two heads' K=64 S^T matmuls concurrently). Softmax exp is split
     across TWO engines: kb%5==2 goes to the DVE as a one-pass Schraudolph
     bit-trick (tensor_scalar mult+add -> int16 = bf16 bits of
     2^((s*SCALE-c)*log2e)), the rest to ACT's exact exp. P@V consumes
     key-block kb-2 (depth-2 software pipeline -- keeps the S matmul of the
     next block from head-of-line blocking behind a P@V that still waits on
     its exp). A ones-column appended to v gives the softmax denominator.
  4. Normalization: oc staged bf16 (fast PSUM release); denominator row
     staged to partition 0 then reciprocal_approx_fast (the custom op
     mishandles base_partition>0); DRAM-bounce partition-broadcast; lazy
     multiply writes oT bf16.
  5. Out-projection bf16, pinned on a covering attention flush so it
     overlaps late attention without stalling the PE queue.
"""

import os
import sys
from contextlib import ExitStack

import numpy as np

for _p in ("/opt/trn_rl_repo",):
    if _p not in sys.path and os.path.isdir(_p):
        sys.path.insert(0, _p)

import concourse.bacc as bacc
import concourse.bass as bass
import concourse.mybir as mybir
import concourse.tile as tile
from concourse.bass_utils import run_bass_kernel_spmd
from concourse.masks import make_identity

F32 = mybir.dt.float32
BF16 = mybir.dt.bfloat16
I16 = mybir.dt.int16
I32 = mybir.dt.int32
AF = mybir.ActivationFunctionType

N_CORES = 8
B, N, D = 4, 2048, 512
H_PER_CORE = 4
DH = 64
GCOLS = H_PER_CORE * DH          # 256 columns per head-group
WCOLS = 3 * GCOLS                # 768 qkv columns per core
SCALE = DH ** -0.5
EPS = 1e-5
P = 128                          # SBUF partitions
NT = N // P                      # 16 token tiles
KT = D // P                      # 4 feature (contraction) tiles
QTW = 512                        # query-slice width for attention
NQT = N // QTW                   # 4 query slices

# Schraudolph bf16-space exp: i16 = round(s*SCALE*log2e*128 + 127*128 - C)
# C centers the one-sided (1+f)/2^f error so DVE-exp'd key blocks are not
# systematically overweighted vs ACT-exp'd ones in the softmax.
EXP_A = float(SCALE * 128.0 / np.log(2.0))
EXP_C = float(os.environ.get("BASS_EXP_C", "7.33"))
EXP_B = float(127 * 128) - EXP_C
# kb % DVE_DIV == DVE_MOD routes that key block's exp to the DVE
DVE_DIV = int(os.environ.get("BASS_DVE_DIV", "5"))
DVE_MOD = int(os.environ.get("BASS_DVE_MOD", "2"))


def _build_nc():
    nc = bacc.Bacc(None)
    x_d = nc.declare_dram_parameter("x", [N, D], F32, isOutput=False)
    wqkv_d = nc.declare_dram_parameter("wqkv", [D, WCOLS], BF16, isOutput=False)
    bqkv_d = nc.declare_dram_parameter("bqkv", [WCOLS, 1], F32, isOutput=False)
    wout_d = nc.declare_dram_parameter("wout", [GCOLS, D], BF16, isOutput=False)
    out_d = nc.declare_dram_parameter("out", [N, D], F32, isOutput=True)

    with tile.TileContext(nc, pool_alloc_mode="queue") as tc, ExitStack() as ctx:
        singles = ctx.enter_context(tc.tile_pool(name="singles", bufs=1))
        xin = ctx.enter_context(tc.tile_pool(name="xin", bufs=6))
        xnp = ctx.enter_context(tc.tile_pool(name="xnp", bufs=4))
        stats = ctx.enter_context(tc.tile_pool(name="stats", bufs=4))
        pP = ctx.enter_context(tc.tile_pool(name="pP", bufs=3))
        pPi = ctx.enter_context(tc.tile_pool(name="pPi", bufs=2))
        smalls = ctx.enter_context(tc.tile_pool(name="smalls", bufs=6))
        rbp = ctx.enter_context(tc.tile_pool(name="rbp", bufs=3))
        outp = ctx.enter_context(tc.tile_pool(name="outp", bufs=3))
        psD = ctx.enter_context(tc.tile_pool(name="psD", bufs=2, space="PSUM"))
        psV = ctx.enter_context(tc.tile_pool(name="psV", bufs=2, space="PSUM"))
        psS = ctx.enter_context(tc.tile_pool(name="psS", bufs=2, space="PSUM"))
        dscr = ctx.enter_context(tc.tile_pool(name="dscr", bufs=6, space="DRAM"))

        identf = singles.tile([P, P], F32)
        make_identity(nc, identf)
        ident = singles.tile([P, P], BF16)
        nc.vector.tensor_copy(out=ident, in_=identf)

        # persistent SBUF tensors
        xT = singles.tile([P, KT, N], BF16)             # xn^T  [feat, token]
        qkT = singles.tile([P, 4, N], BF16)             # [qT(2 tiles), kT(2 tiles)]
        v_aug = singles.tile([P, NT, H_PER_CORE, DH + 1], BF16)
        oT = singles.tile([P, 2, N], BF16)              # O^T rows (4 heads x 64)
        w_sb = singles.tile([P, KT, WCOLS], BF16)
        bias_sb = singles.tile([P, 6], F32)
        oc_all = singles.tile([DH, NQT * H_PER_CORE, QTW], BF16)
        wout_sb = singles.tile([P, 2, D], BF16)

        # ones columns of v_aug
        ones_sb = singles.tile([P, 1], F32)
        nc.vector.memset(ones_sb, 1.0)
        nc.vector.tensor_copy(
            out=v_aug[:, :, :, DH : DH + 1],
            in_=ones_sb.to_broadcast((P, NT, H_PER_CORE, 1)),
        )

        # PE matmuls accept only ONE sync wait command. Sacrificial ldweights
        # ops (no PSUM output, single dependency each) make the PE observe
        # fresh semaphore ticks so real matmuls keep to one wait.
        def pe_observe(ap):
            nc.tensor.ldweights(ap.bitcast(BF16))

        pe_observe(ident[:, 0:1])

        # ---- q/k projection group builder (used by phases A/B/C) -----------
        def qk_group_steps(mi, nt):
            ps = psD.tile([P, QTW], F32, tag="ps")
            for kt in range(KT):
                yield lambda kt=kt, ps=ps: nc.tensor.matmul(
                    ps,
                    w_sb[:, kt, mi * P : (mi + 1) * P],
                    xT[:, kt, nt * QTW : (nt + 1) * QTW],
                    start=(kt == 0),
                    stop=(kt == KT - 1),
                )

            def bias_and_observe(ps=ps, mi=mi, nt=nt):
                nc.vector.tensor_scalar(
                    out=qkT[:, mi, nt * QTW : (nt + 1) * QTW],
                    in0=ps,
                    scalar1=bias_sb[:, mi : mi + 1],
                    scalar2=None,
                    op0=mybir.AluOpType.add,
                )
                pe_observe(qkT[:, mi, (nt + 1) * QTW - 1 : (nt + 1) * QTW])

            yield bias_and_observe

        # ---- Phase B/C machinery (defined early: phase A pumps attention) --
        def steps_of(groups):
            for mi, nt in groups:
                yield from qk_group_steps(mi, nt)

        _deferred = steps_of([])  # all groups run in phase A PE slack

        # group (mi, nt) only needs xT tokens [nt*512, (nt+1)*512), complete
        # after tile 4nt+3: spread all 16 groups across phase A
        QK_SCHED = {
            3: [(0, 0), (2, 0)], 4: [(3, 0)], 5: [(1, 0)],
            7: [(2, 1), (0, 1)], 8: [(3, 1)], 9: [(1, 1)],
            11: [(2, 2), (0, 2)], 12: [(3, 2)], 13: [(1, 2)],
            15: [(2, 3), (0, 3), (3, 3), (1, 3)],
        }

        last_flush = {}
        norm_pending = []
        phase_a_done = [False]

        def make_norm_chain(hp, qt, h0, h1, po0, po1, qs):
            def chain():
                # normalize both heads: bf16 staging copy + fast reciprocal
                # release the PSUM slots promptly; the DMA-broadcast/mul
                # chain lags by design and only gates phase D.
                for h, po in ((h0, po0), (h1, po1)):
                    u = h * NQT + qt
                    r0 = (h % 2) * DH
                    nc.vector.tensor_copy(out=oc_all[:, u, :], in_=po[0:DH, :])
                    den = smalls.tile([1, QTW], F32)
                    nc.vector.tensor_copy(out=den, in_=po[DH : DH + 1, :])
                    r = smalls.tile([1, QTW], F32)
                    nc.vector.reciprocal_approx_fast(out=r, in_=den)
                    rd = dscr.tile([1, QTW], F32)
                    nc.sync.dma_start(out=rd, in_=r)
                    rb = rbp.tile([DH, QTW], F32)
                    nc.sync.dma_start(out=rb, in_=rd.to_broadcast((DH, QTW)))
                    nc.vector.tensor_mul(
                        out=oT[r0 : r0 + DH, h // 2, qs],
                        in0=oc_all[:, u, :],
                        in1=rb,
                    )

            return chain

        # attention (S^T layout, head-paired), as a generator yielding after
        # each key-block iteration so phase A can pump unit (0,0) early
        def attention_gen():
            for hp in range(2):
                h0, h1 = 2 * hp, 2 * hp + 1
                mi_q, mi_k = hp, 2 + hp
                for qt in range(NQT):
                    qs = slice(qt * QTW, (qt + 1) * QTW)
                    po0 = psV.tile([DH + 1, QTW], F32, tag="po")
                    po1 = psV.tile([DH + 1, QTW], F32, tag="po")
                    pend = []

                    def pv_flush(pkb, ppT, stop, hp=hp, h0=h0, h1=h1,
                                 po0=po0, po1=po1):
                        if pkb == 0:
                            pe_observe(ppT[:, 0, 0:1])
                        nc.tensor.matmul(
                            po0, v_aug[:, pkb, h0, :], ppT[:, 0, :],
                            start=(pkb == 0), stop=stop,
                        )
                        mm = nc.tensor.matmul(
                            po1, v_aug[:, pkb, h1, :], ppT[:, 1, :],
                            start=(pkb == 0), stop=stop,
                        )
                        if hp == 0 and phase_a_done[0]:
                            for _ in range(2):
                                step = next(_deferred, None)
                                if step is not None:
                                    step()
                        return mm

                    for kb in range(NT):
                        if kb == 1 and norm_pending:
                            # previous unit's normalize chain, deferred past
                            # the unit boundary so its DVE ops don't delay
                            # this unit's pipeline ramp
                            norm_pending.pop(0)()
                        ks = slice(kb * P, (kb + 1) * P)
                        ps_s = psS.tile([P, 2, QTW], F32, tag="s")
                        nc.tensor.matmul(
                            ps_s[:, 0, :],
                            qkT[0:DH, mi_k, ks],
                            qkT[0:DH, mi_q, qs],
                            start=True,
                            stop=True,
                        )
                        nc.tensor.matmul(
                            ps_s[:, 1, :],
                            qkT[DH:P, mi_k, ks],
                            qkT[DH:P, mi_q, qs],
                            start=True,
                            stop=True,
                        )
                        if kb % DVE_DIV == DVE_MOD:
                            # DVE Schraudolph: i16 bits of 2^((s*SCALE-c)*log2e)
                            pTi = pPi.tile([P, 2, QTW], I16)
                            nc.vector.tensor_scalar(
                                out=pTi,
                                in0=ps_s,
                                scalar1=EXP_A,
                                scalar2=EXP_B,
                                op0=mybir.AluOpType.mult,
                                op1=mybir.AluOpType.add,
                            )
                            pT = pTi.bitcast(BF16)
                        else:
                            pT = pP.tile([P, 2, QTW], BF16)
                            nc.scalar.activation(
                                out=pT, in_=ps_s, func=AF.Exp, scale=SCALE
                            )
                        if len(pend) == 2:
                            pkb, ppT = pend.pop(0)
                            pv_flush(pkb, ppT, stop=False)
                        pend.append((kb, pT))
                        yield
                    pv_flush(*pend.pop(0), stop=False)
                    last_att_mm = pv_flush(*pend.pop(0), stop=True)
                    if hp == 1:
                        last_flush[qt] = last_att_mm
                    norm_pending.append(
                        make_norm_chain(hp, qt, h0, h1, po0, po1, qs)
                    )

        _att = attention_gen()

        def pump(n):
            for _ in range(n):
                if next(_att, "END") == "END":
                    break

        # ---- Phase A: LayerNorm + transpose + v projection -----------------
        for tt in range(NT):
            x_tile = xin.tile([P, D], F32)
            nc.sync.dma_start(out=x_tile, in_=x_d[tt * P : (tt + 1) * P, :])
            if tt == 0:
                # weight DMAs issued after the first x tile so their
                # descriptor generation doesn't delay the LayerNorm start;
                # split per kt so no single queue carries the full 768KB
                for kt in range(KT):
                    nc.sync.dma_start(
                        out=w_sb[:, kt, :],
                        in_=wqkv_d[kt * P : (kt + 1) * P, :],
                    )
                nc.sync.dma_start(
                    out=bias_sb, in_=bqkv_d[:, :].rearrange("(t p) o -> p (t o)", p=P)
                )
                for ki in range(2):
                    nc.sync.dma_start(
                        out=wout_sb[:, ki, :],
                        in_=wout_d[ki * P : (ki + 1) * P, :],
                    )
                for kt in range(KT):
                    pe_observe(w_sb[:, kt, 2 * GCOLS : 2 * GCOLS + 1])
                pe_observe(wout_sb[:, 0, 0:1])
                pe_observe(wout_sb[:, 1, 0:1])
            st = stats.tile([P, nc.vector.BN_STATS_DIM], F32)
            nc.vector.bn_stats(out=st, in_=x_tile)
            mv = stats.tile([P, nc.vector.BN_AGGR_DIM], F32)
            nc.vector.bn_aggr(out=mv, in_=st)
            # rstd = rsqrt(var+eps) entirely on the DVE (quake seed + one
            # Newton step, ~0.2% -- under the bf16 rounding already applied
            # to xn). Keeping Sqrt off ACT means phase A only uses Copy/Exp,
            # all in ACT's exp_and_others table set: interleaving the pumped
            # attention exps causes no ~2.9us ACT_TABLE_LOAD ping-pong.
            ve = stats.tile([P, 1], F32)
            nc.vector.tensor_scalar(
                out=ve, in0=mv[:, 1:2], scalar1=EPS, scalar2=None,
                op0=mybir.AluOpType.add,
            )
            y0i = stats.tile([P, 1], I32)
            nc.vector.tensor_scalar(
                out=y0i, in0=ve.bitcast(I32), scalar1=1, scalar2=None,
                op0=mybir.AluOpType.logical_shift_right,
            )
            nc.vector.tensor_scalar(
                out=y0i, in0=y0i, scalar1=-1, scalar2=0x5F3759DF,
                op0=mybir.AluOpType.mult, op1=mybir.AluOpType.add,
            )
            y0 = y0i.bitcast(F32)
            t1 = stats.tile([P, 1], F32)
            nc.vector.tensor_mul(out=t1, in0=y0, in1=y0)
            nc.vector.tensor_mul(out=t1, in0=t1, in1=ve)
            nc.vector.tensor_scalar(
                out=t1, in0=t1, scalar1=-0.5, scalar2=1.5,
                op0=mybir.AluOpType.mult, op1=mybir.AluOpType.add,
            )
            y1 = stats.tile([P, 1], F32)
            nc.vector.tensor_mul(out=y1, in0=y0, in1=t1)
            t2 = stats.tile([P, 1], F32)
            nc.vector.tensor_mul(out=t2, in0=y1, in1=y1)
            nc.vector.tensor_mul(out=t2, in0=t2, in1=ve)
            nc.vector.tensor_scalar(
                out=t2, in0=t2, scalar1=-0.5, scalar2=1.5,
                op0=mybir.AluOpType.mult, op1=mybir.AluOpType.add,
            )
            rstd = stats.tile([P, 1], F32)
            nc.vector.tensor_mul(out=rstd, in0=y1, in1=t2)
            xn = xnp.tile([P, D], BF16)
            nc.vector.tensor_scalar(
                out=xn,
                in0=x_tile,
                scalar1=mv[:, 0:1],
                scalar2=rstd,
                op0=mybir.AluOpType.subtract,
                op1=mybir.AluOpType.mult,
            )
            # 4 bf16 transposes into ONE PSUM bank (borrowing the S pool,
            # idle in phase A), one ACT copy to xT
            ps4 = psS.tile([P, KT, P], BF16, tag="s")
            for ft in range(KT):
                nc.tensor.transpose(
                    ps4[:, ft, :], xn[:, ft * P : (ft + 1) * P], ident
                )
            nc.scalar.copy(
                out=xT[:, :, tt * P : (tt + 1) * P].rearrange("p k t -> p k t"),
                in_=ps4,
            )
            pe_observe(xT[:, KT - 1, tt * P + P - 1 : tt * P + P])
            # v projection (no bias: folded into the host epilogue)
            ps = psD.tile([P, GCOLS], F32, tag="ps")
            for kt in range(KT):
                nc.tensor.matmul(
                    ps,
                    xT[:, kt, tt * P : (tt + 1) * P],
                    w_sb[:, kt, 2 * GCOLS : 3 * GCOLS],
                    start=(kt == 0),
                    stop=(kt == KT - 1),
                )
            nc.vector.tensor_copy(
                out=v_aug[:, tt, :, 0:DH],
                in_=ps.rearrange("p (h d) -> p h d", h=H_PER_CORE),
            )
            # q/k groups interleaved into phase A PE slack
            for mi, nt in QK_SCHED.get(tt, ()):
                for step in qk_group_steps(mi, nt):
                    step()
            # pump one attention iteration of unit (0,0) per tile: kb=tt-4
            # only needs k(nt=(tt-4)//4) (ready at tt>=4nt+3) and v_aug
            # tiles <= tt-4, both already emitted
            if tt >= 4:
                pump(1)

        # ---- Phase C: finish the attention stream --------------------------
        phase_a_done[0] = True
        for _ in _att:
            pass

        while norm_pending:
            norm_pending.pop(0)()

        # Keep the PE HAM-warm across the normalize-chain tail.
        for wk in range(8):
            ps = psD.tile([P, QTW], F32, tag="ps")
            nc.tensor.matmul(
                ps,
                qkT[0:DH, 0, 0:P],
                qkT[0:DH, 0, 0:QTW],
                start=True,
                stop=True,
            )

        # PE observes the final oT write tick before the out-projection
        pe_observe(oT[0:DH, 1, N - 1 : N])

        # ---- Phase D: out projection -----------------------------------
        ob_hist = []
        for tt in range(NT):
            if len(ob_hist) >= 2:
                pe_observe(ob_hist[-2][:, 0:1])
            ps = psD.tile([P, D], F32, tag="ps")
            for ki in range(2):
                mm = nc.tensor.matmul(
                    ps,
                    oT[:, ki, tt * P : (tt + 1) * P],
                    wout_sb[:, ki, :],
                    start=(ki == 0),
                    stop=(ki == 1),
                )
                pin = last_flush[min(tt // NQT + 1, NQT - 1)]
                tile.add_dep_helper(
                    mm.ins, pin.ins, sync=False,
                    reason="phase D after covering attention flush",
                )
            ob = outp.tile([P, D], F32)
            nc.vector.tensor_copy(out=ob, in_=ps)
            ob_hist.append(ob)
            nc.sync.dma_start(out=out_d[tt * P : (tt + 1) * P, :], in_=ob)

    nc.compile()
    return nc


_NC_CACHE = {}
last_results = None  # BassKernelResults of the most recent run (for test.py)


def _get_nc():
    key = (EXP_C, DVE_DIV, DVE_MOD)
    if key not in _NC_CACHE:
        _NC_CACHE[key] = _build_nc()
    return _NC_CACHE[key]


def _to_bf16(a):
    import ml_dtypes

    return np.ascontiguousarray(a.astype(ml_dtypes.bfloat16))


def kernel(x, gamma, beta, w_qkv, w_out):
    global last_results
    x = np.ascontiguousarray(np.asarray(x, dtype=np.float32))
    gamma = np.asarray(gamma, dtype=np.float32)
    beta = np.asarray(beta, dtype=np.float32)
    w_qkv = np.asarray(w_qkv, dtype=np.float32)
    w_out = np.asarray(w_out, dtype=np.float32)

    # fold gamma/beta into the projection (exact algebra)
    wp = gamma[:, None] * w_qkv                      # [512, 1536]
    bp = beta @ w_qkv                                # [1536]

    in_maps = []
    for c in range(N_CORES):
        b = c // 2
        g = c % 2
        sl = [slice(s * D + g * GCOLS, s * D + (g + 1) * GCOLS) for s in range(3)]
        wg = np.concatenate([wp[:, s] for s in sl], axis=1)          # [512, 768]
        bg = np.concatenate([bp[s] for s in sl])[:, None]            # [768, 1]
        wo = w_out[g * GCOLS : (g + 1) * GCOLS, :]                   # [256, 512]
        in_maps.append(
            {
                "x": np.ascontiguousarray(x[b]),
                "wqkv": _to_bf16(wg),
                "bqkv": np.ascontiguousarray(bg.astype(np.float32)),
                "wout": _to_bf16(wo),
            }
        )

    nc = _get_nc()
    last_results = run_bass_kernel_spmd(nc, in_maps, list(range(N_CORES)))
    outs = [m["out"] for m in last_results.results]
    # v-bias epilogue: P@(V + 1 b^T) = P@V + den*b^T, so after normalization
    # the correction is simply +b_v @ w_out on every token row (exact).
    vb_corr = (bp[2 * D :] @ w_out).astype(np.float32)   # [512]
    out = np.stack([outs[2 * b] + outs[2 * b + 1] + vb_corr for b in range(B)])
    return np.ascontiguousarray(out.astype(np.float32))
